# revision 50
# baseline (speedup 1.0000x reference)
"""Trainium2 Bass kernel for 16-head causal MHA (RMSNorm+RoPE on q,k).

Tensor-parallel over heads: 8 cores x 2 heads each. Each core computes
qkv projection for its heads, norm+rope, causal attention, and a partial
out-projection; the host sums the 8 partial outputs.

v4 notes (on top of the v2 exp-stationary PV design):
- Scores are computed transposed [k, q]; exp tiles serve as the
  STATIONARY matmul operand for PV with a ones-column appended to V, so
  the PV output lands as [q, v|den]: softmax denominator is column 128
  and the division is a per-partition scalar multiply.
- Single fused pipeline: attention group g (q rows [512g, 512g+512)) is
  emitted interleaved with phase-1 tiles >= 4g+4, so PE never drains at
  the phase boundary and the exp/ACT load spreads over the whole span.
- ACT runs ONLY Exp + table-free copies (Square/Sqrt thrash the exp
  activation table at ~1.3us per reload). The RMSNorm 1/sqrt(m) runs on
  DVE as a 2-step Newton iteration seeded with the Taylor line
  1.5 - m/2 (m = mean-square concentrates near 1), batched per 2 tiles.
- The k-side norm scale commutes through RoPE and folds into the exp's
  per-partition scale AP, so only q is scaled explicitly.
- PSUM budget (8 banks): p_qk 2 + p_v 1 + {scores, p_y, transposes}
  shared 3-buf tag 3 + p_o 2.
"""
import os
import ml_dtypes
import numpy as np

import concourse.bacc as bacc
import concourse.mybir as mybir
import concourse.tile as tile
from concourse.ap import AP
from concourse.bass_utils import run_bass_kernel_spmd


def _bcast_mid(ap2d, n):
    """[128, X] -> [128, n, X] with step-0 middle dim."""
    return AP(tensor=ap2d.tensor, offset=ap2d.offset,
              ap=[list(ap2d.ap[0]), [0, n], list(ap2d.ap[1])])

F32 = mybir.dt.float32
BF16 = mybir.dt.bfloat16
WDT = BF16
AF = mybir.ActivationFunctionType
ALU = mybir.AluOpType
AX = mybir.AxisListType

N_CORES = 8
L = 2048
D = 2048
HD = 128
N_HEAD = 16
HPC = N_HEAD // N_CORES  # heads per core = 2
LT = 128                 # L-tile rows
NT = L // LT             # 16 L-tiles
HC = 128                 # hid chunk
NHC = D // HC            # 16 hid chunks
QT = 512                 # q-tile width in attention
NQT = L // QT            # 4
VW = 130                 # v row stride (128 dims + ones col + pad)
EPS = 1e-5
ROPE_BASE = 10000.0
SCALE = 1.0 / float(np.sqrt(HD))
NEG = -1.0e9
MSCALE = 2.75            # rsqrt range normalizer: m*MSCALE lands in [0.5, 2]
INTERLEAVE = os.environ.get("MHA_INTERLEAVE", "1") == "1"
NEWTON = os.environ.get("MHA_NEWTON", "1") == "1"
TORDER = list(range(NT))
# attention q-groups (first q-tile, n q-tiles): g0-g2 full 512-wide; the
# tail group is split in two 256-wide halves so the first half unblocks
# at tile pair (12,13) and the serial tail is only q-tiles 14-15.
GROUPS = [(0, 4), (4, 4), (8, 4), (12, 2), (14, 2)]


def build():
    nc = bacc.Bacc("TRN2", target_bir_lowering=False, debug=False,
                   enable_asserts=False, num_devices=N_CORES)

    # Per-core external inputs (host-prepped layouts; see prep_inputs()).
    xt = nc.dram_tensor("xt", [NT, HC, NHC, LT], WDT, kind="ExternalInput")
    wt = nc.dram_tensor("wt", [D, 6 * HD], WDT, kind="ExternalInput")
    wo = nc.dram_tensor("wo", [HD, HPC, D], WDT, kind="ExternalInput")
    w1 = nc.dram_tensor("w1", [LT, NT, HD], F32, kind="ExternalInput")
    w2 = nc.dram_tensor("w2", [LT, NT, HD], F32, kind="ExternalInput")
    mask4 = nc.dram_tensor("mask4", [128, 128], F32, kind="ExternalInput")
    ident_in = nc.dram_tensor("ident", [128, 128], WDT, kind="ExternalInput")

    # partial outputs travel bf16: host-side sum of 8 bf16 partials adds
    # ~2.3e-3 rel err (tolerance 2e-2) and halves evac + output DMA cost
    out = nc.dram_tensor("out", [L, D], WDT, kind="ExternalOutput")

    with tile.TileContext(nc) as tc:
        with (
            tc.tile_pool(name="const", bufs=1) as constp,
            tc.tile_pool(name="wpool", bufs=1) as wpool,
            tc.tile_pool(name="persist", bufs=1) as persist,
            tc.tile_pool(name="xin", bufs=7) as xin,
            tc.tile_pool(name="qkv", bufs=3) as qkvp,
            tc.tile_pool(name="attn", bufs=4) as attnp,
            tc.tile_pool(name="res", bufs=4) as resp,
            tc.tile_pool(name="ps_qkv", bufs=1, space="PSUM") as ps_pv,
            tc.tile_pool(name="ps_s", bufs=3, space="PSUM") as ps_sc,
            tc.tile_pool(name="ps_o", bufs=1, space="PSUM") as ps_po,
        ):
            # ---- startup DMA order: first matmul needs x tile 0 + w
            # chunk 0 only; issue those first, stream the rest behind.
            w_sb = wpool.tile([128, NHC, 6 * HD], WDT)
            x_first = xin.tile([128, NHC, LT], WDT, tag="x", name="x_tile")
            nc.sync.dma_start(out=x_first, in_=xt[0, :, :, :])
            for c in range(4):
                eng = nc.scalar if c % 2 == 0 else nc.sync
                eng.dma_start(out=w_sb[:, c, :],
                              in_=wt[c * 128:(c + 1) * 128, :])
            w1_sb = constp.tile([128, NT, HD], F32)
            w2_sb = constp.tile([128, NT, HD], F32)
            nc.scalar.dma_start(out=w1_sb[:, 0:2, :], in_=w1[:, 0:2, :])
            nc.sync.dma_start(out=w2_sb[:, 0:2, :], in_=w2[:, 0:2, :])
            x_1 = xin.tile([128, NHC, LT], WDT, tag="x", name="x_tile")
            nc.sync.dma_start(out=x_1, in_=xt[1, :, :, :])
            x_2 = xin.tile([128, NHC, LT], WDT, tag="x", name="x_tile")
            nc.scalar.dma_start(out=x_2, in_=xt[2, :, :, :])
            for c in range(4, NHC):
                eng = nc.scalar if c % 2 == 0 else nc.sync
                eng.dma_start(out=w_sb[:, c, :],
                              in_=wt[c * 128:(c + 1) * 128, :])
            nc.scalar.dma_start(out=w1_sb[:, 2:6, :], in_=w1[:, 2:6, :])
            nc.sync.dma_start(out=w2_sb[:, 2:6, :], in_=w2[:, 2:6, :])
            mask_sb = constp.tile([128, 128], F32)
            nc.gpsimd.dma_start(out=mask_sb, in_=mask4[:, :])
            ident = constp.tile([128, 128], WDT)
            nc.gpsimd.dma_start(out=ident, in_=ident_in[:, :])
            wo_sb = wpool.tile([128, HPC, D], WDT)
            eps_sb = constp.tile([128, 1], F32)
            nc.vector.memset(eps_sb, EPS)

            # persistent activations
            # v_sb: [kpos-part, t, head, 130]; col 128 is the ones column
            # feeding the softmax denominator, col 129 is alignment pad.
            v_sb = persist.tile([128, NT, HPC, VW], WDT)
            nc.vector.memset(v_sb[:, :, :, 128:VW], 1.0)
            qT = persist.tile([128, HPC, L], WDT)               # [d, h, L]
            kT = persist.tile([128, HPC, L], WDT)
            # per-kpos exp scale: SCALE / sqrt(mean k^2 + eps), per head
            sk_sb = persist.tile([128, NT, HPC], F32)



            xs = {0: x_first, 1: x_1, 2: x_2}

            def load_x(t):
                x_tl = xin.tile([128, NHC, LT], WDT, tag="x", name="x_tile")
                nc.gpsimd.dma_start(out=x_tl, in_=xt[t, :, :, :])
                xs[t] = x_tl

            for i in range(3, 6):
                load_x(TORDER[i])

            # ---------------- phase 1 unit generator ------------------
            pair_state = {}

            def phase1_units(idx, t):
                x_tile = xs.pop(t)
                p_qk = ps_pv.tile([128, 4 * HD], F32, tag="pqk", bufs=2,
                                  name="p_qk")
                p_v = ps_pv.tile([128, HPC * HD], F32, tag="pv", bufs=1,
                                 name="p_v")

                def chunk(c):
                    nc.tensor.matmul(p_qk, x_tile[:, c, :],
                                     w_sb[:, c, 0:4 * HD],
                                     start=(c == 0), stop=(c == NHC - 1))
                    nc.tensor.matmul(p_v, x_tile[:, c, :],
                                     w_sb[:, c, 4 * HD:6 * HD],
                                     start=(c == 0), stop=(c == NHC - 1))
                for c in range(NHC):
                    yield lambda c=c: chunk(c)

                def post_a():
                    if idx == 1:
                        nc.scalar.dma_start(out=w1_sb[:, 6:NT, :],
                                            in_=w1[:, 6:NT, :])
                        nc.sync.dma_start(out=w2_sb[:, 6:NT, :],
                                          in_=w2[:, 6:NT, :])
                    if idx == 4:
                        nc.scalar.dma_start(out=wo_sb, in_=wo[:, :, :])
                    if idx + 6 < NT:
                        load_x(TORDER[idx + 6])
                    # v out of psum (ACT copy: table-free)
                    nc.scalar.copy(
                        v_sb[:, t, :, 0:HD],
                        p_v.rearrange("p (h d) -> p h d", h=HPC))
                    # rope RAW q,k straight out of PSUM (norm commutes
                    # through rope); fp32 in, bf16 out, de-interleaved
                    roped = qkvp.tile([128, 4 * HD], WDT, tag="roped",
                                      bufs=4, name="roped")
                    roped4 = roped.rearrange("p (g h x) -> p g h x", g=4, h=2)
                    for half, wtab in ((0, w1_sb), (1, w2_sb)):
                        z = qkvp.tile([128, 4 * HD], F32, tag="z", name="z")
                        nc.vector.tensor_mul(
                            z.rearrange("p (g d) -> p g d", g=4),
                            p_qk.rearrange("p (g d) -> p g d", g=4),
                            _bcast_mid(wtab[:, t, :], 4))
                        with nc.allow_low_precision("2-elem rope pairs"):
                            nc.vector.reduce_sum(
                                roped4[:, :, half, :],
                                z.rearrange("p (g x two) -> p g x two",
                                            g=4, two=2),
                                axis=AX.X)
                    if NEWTON:
                        # ssum[g] = MSCALE*mean_d roped[g,d]^2 (rope
                        # preserves row norms); batched per tile pair
                        if t % 2 == 0:
                            ss = qkvp.tile([128, 8], F32, tag="ssum",
                                           name="ssum")
                            pair_state['ssum'] = ss
                        ss = pair_state['ssum']
                        off = (t % 2) * 4
                        sq = qkvp.tile([128, 4 * HD], WDT, tag="sq",
                                       name="sq")
                        nc.vector.tensor_mul(sq, roped, roped)
                        nc.vector.reduce_sum(
                            ss[:, off:off + 4],
                            sq.rearrange("p (g d) -> p g d", g=4), axis=AX.X)
                        nc.vector.tensor_scalar_mul(
                            ss[:, off:off + 4], ss[:, off:off + 4],
                            MSCALE / HD)
                    else:
                        # baseline norm chain: ACT square + DVE reduce +
                        # ACT sqrt(+eps) + DVE recip, then scale+transpose
                        sq = qkvp.tile([128, 4 * HD], F32, tag="sq",
                                       name="sq")
                        nc.scalar.activation(sq, roped, AF.Square)
                        ssb = qkvp.tile([128, 4], F32, tag="ssb", name="ssb")
                        nc.vector.reduce_sum(
                            ssb, sq.rearrange("p (g d) -> p g d", g=4),
                            axis=AX.X)
                        nc.scalar.activation(ssb, ssb, AF.Sqrt,
                                             scale=1.0 / HD, bias=eps_sb)
                        s_val = qkvp.tile([128, 4], F32, tag="sval",
                                          name="s_val")
                        nc.vector.reciprocal(s_val, ssb)
                        finish_tile(t, roped, s_val, 0)
                    pair_state[idx % 2] = roped
                yield post_a

                def finish_tile(tt, roped, y, soff):
                    # scale q,k segs by 1/sqrt(m) in place (DVE); the PE
                    # transposes are deferred into the next tile's stream
                    # so they don't head-of-line block behind this chain
                    roped2 = roped.rearrange("p (g d) -> p g d", g=4)
                    nc.vector.tensor_mul(
                        roped2, roped2,
                        y[:, soff:soff + 4].to_broadcast([128, 4, HD]))

                    def tp_unit(seg):
                        tgt = qT if seg < 2 else kT
                        h = seg % 2
                        p_tr = ps_sc.tile([128, 128], WDT, tag="sc",
                                          name="p_tr")
                        nc.tensor.transpose(
                            p_tr, roped[:, seg * HD:(seg + 1) * HD], ident)
                        nc.scalar.copy(tgt[:, h, tt * LT:(tt + 1) * LT],
                                       p_tr)
                    pair_state.setdefault('tp', []).extend(
                        (lambda seg=seg: tp_unit(seg)) for seg in range(4))

                if NEWTON and t % 2 == 1:
                    def newton():
                        # DVE rsqrt for the pair: ss holds m' = MSCALE*m
                        # (m' in ~[0.5, 2.0]); seed y0 = (1 + 1/m')/2
                        # (<=10% err there), 2 Newton steps -> <=4e-4;
                        # the sqrt(MSCALE) unscale folds into the last mul.
                        # Single-scalar DVE ops only (2-imm tensor_scalar /
                        # scalar_tensor_tensor mis-execute on HW).
                        ss = pair_state['ssum']
                        y = qkvp.tile([128, 8], F32, tag="rs_y", name="y_rs")
                        r = qkvp.tile([128, 8], F32, tag="rs_r", name="r_rs")
                        nc.vector.reciprocal(r, ss)
                        nc.vector.tensor_scalar_add(r, r, 1.0)
                        nc.vector.tensor_scalar_mul(y, r, 0.5)
                        for it in range(2):
                            u = qkvp.tile([128, 8], F32, tag="rs_u",
                                          name="u_rs")
                            nc.vector.tensor_mul(u, y, y)
                            nc.vector.tensor_mul(u, u, ss)
                            nc.vector.tensor_scalar_mul(u, u, -0.5)
                            nc.vector.tensor_scalar_add(u, u, 1.5)
                            nc.vector.tensor_mul(y, y, u)
                        nc.vector.tensor_scalar_mul(y, y, float(MSCALE ** 0.5))
                        return y
                    holder = {}

                    def post_b0():
                        y = holder.setdefault('y', newton())
                        finish_tile(t - 1, pair_state[0], y, 0)

                    def post_b1():
                        y = holder['y']
                        finish_tile(t, pair_state[1], y, 4)
                    yield post_b0
                    yield post_b1

            # --------------- attention unit generator -----------------
            # p_o packs the group's q-chunks of [v(128)|den(1)] into PSUM.
            # The first chunk of each bank carries start=True (clears the
            # bank's has_written bits); later chunks in the same bank
            # overwrite-on-clear-bit at kc==0.
            PO_OFF = (0, 130, 260, 512)
            PO_START = (0, 3)  # qc's whose kc==0 matmul is the bank-first

            def head_units(t0, ntq, h, oT_tiles):
                """Yields (need_tiles, unit) for q rows [128*t0, ...)."""
                W = ntq * 128
                nkc = t0 + ntq
                qtiles = frozenset(range(t0, t0 + ntq))
                p_o = ps_po.tile([128, 1024], F32, tag="po", name="p_o")
                exps = {}

                def score(kc):
                    diag = kc >= t0
                    q0 = (kc - t0) * 128 if diag else 0
                    p_s = ps_sc.tile([128, QT], F32, tag="sc", name="p_s")
                    nc.tensor.matmul(
                        p_s[:, q0:W], kT[:, h, kc * 128:(kc + 1) * 128],
                        qT[:, h, t0 * 128 + q0:t0 * 128 + W],
                        start=True, stop=True)
                    if diag:
                        nc.vector.tensor_add(
                            p_s[:, q0:q0 + 128], p_s[:, q0:q0 + 128], mask_sb)
                    expT = attnp.tile([128, QT], WDT, tag="expT", bufs=6,
                                      name="expT")
                    nc.scalar.activation(expT[:, q0:W], p_s[:, q0:W],
                                         AF.Exp, scale=SCALE)
                    exps[kc] = expT

                def pv(kc):
                    expT = exps.pop(kc)
                    for qc in range(max(0, kc - t0), ntq):
                        off = PO_OFF[qc]
                        nc.tensor.matmul(
                            p_o[:, off:off + 129],
                            expT[:, qc * 128:(qc + 1) * 128],
                            v_sb[:, kc, h, 0:129],
                            start=(kc == 0 and qc in PO_START),
                            stop=(kc == t0 + qc),
                            skip_group_check=True)

                yield qtiles | {0}, lambda: score(0)
                for kc in range(1, nkc):
                    yield (qtiles | {kc},
                           lambda kc=kc: (score(kc), pv(kc - 1)))
                yield qtiles, lambda: pv(nkc - 1)

                # o_sb[q, vd] = p_o[q, 0:128] / den (den = col 128), then
                # transpose back to [vd, q] on PE for the out-projection
                def division(qc):
                    off = PO_OFF[qc]
                    inv = attnp.tile([128, 1], F32, tag="inv", bufs=4,
                                     name="inv")
                    nc.vector.reciprocal(inv, p_o[:, off + 128:off + 129])
                    o_sb = attnp.tile([128, HD], WDT, tag="osb", bufs=8,
                                      name="o_sb")
                    nc.vector.tensor_scalar_mul(o_sb, p_o[:, off:off + HD],
                                                inv)
                    p_tr = ps_sc.tile([128, 128], WDT, tag="sc", name="p_tr2")
                    nc.tensor.transpose(p_tr, o_sb, ident)
                    oT = resp.tile([128, 128], WDT, tag="oT", bufs=40,
                                   name="oT")
                    nc.vector.tensor_copy(oT, p_tr)
                    oT_tiles[(h, qc)] = oT
                for qc in range(ntq):
                    yield qtiles, lambda qc=qc: division(qc)

            def outproj_units(t0, ntq, oT_tiles):
                for tt in range(ntq):
                    t = t0 + tt
                    for ec in range(4):
                        def u(t=t, tt=tt, ec=ec):
                            p_y = ps_sc.tile([128, QT], F32, tag="sc",
                                             name="p_y")
                            nc.tensor.matmul(
                                p_y, oT_tiles[(0, tt)],
                                wo_sb[:, 0, ec * 512:(ec + 1) * 512],
                                start=True, stop=False)
                            nc.tensor.matmul(
                                p_y, oT_tiles[(1, tt)],
                                wo_sb[:, 1, ec * 512:(ec + 1) * 512],
                                start=False, stop=True)
                            y = resp.tile([128, QT], WDT, tag="y", bufs=4,
                                          name="y")
                            nc.vector.tensor_copy(y, p_y)
                            eng = nc.gpsimd if ec % 2 == 0 else nc.sync
                            eng.dma_start(
                                out=out[t * LT:(t + 1) * LT,
                                        ec * 512:(ec + 1) * 512],
                                in_=y)
                        yield u

            def attention_units():
                """Yields (need_tiles, unit). Out-proj units of early
                groups are DEFERRED and woven into the later (exp-bound,
                ACT-limited) groups so PE stays dense there."""
                backlog = []
                for gi, (t0, ntq) in enumerate(GROUPS):
                    oT_tiles = {}
                    last = gi == len(GROUPS) - 1
                    # for the last group, weave its own out-proj directly
                    # behind each h1 division so the tail doesn't serialize
                    op3 = (list(outproj_units(t0, ntq, oT_tiles))
                           if last else None)
                    for h in range(HPC):
                        hu = list(head_units(t0, ntq, h, oT_tiles))
                        nd = len(hu) - ntq  # index of first division unit
                        for i, (need, u) in enumerate(hu):
                            yield need, u
                            if last and h == 1 and i >= nd:
                                for uu in op3[(i - nd) * 4:(i - nd + 1) * 4]:
                                    yield frozenset(), uu
                            elif gi >= 2 and backlog:
                                yield frozenset(), backlog.pop(0)
                    if not last:
                        backlog.extend(outproj_units(t0, ntq, oT_tiles))
                for n in backlog:
                    yield frozenset(), n

            # ---------------- fused emission schedule -----------------
            attn = attention_units()
            pending = None  # (need, unit) fetched but not yet eligible
            tiles_done = set()

            def pump_attention(budget):
                """Emit up to `budget` eligible attention units."""
                nonlocal pending
                emitted = 0
                while emitted < budget:
                    if pending is None:
                        pending = next(attn, None)
                        if pending is None:
                            return emitted
                    need, u = pending
                    if not need <= tiles_done:
                        return emitted  # inputs not emitted yet
                    pending = None
                    u()
                    emitted += 1
                return emitted

            deferred = []     # previous pair's transpose units
            pending_pair = set()
            for idx, t in enumerate(TORDER):
                units = list(phase1_units(idx, t))
                for u in units:
                    u()
                    if deferred:
                        deferred.pop(0)()
                        if not deferred:
                            # pair fully emitted (incl. qT/kT transposes)
                            tiles_done.update(pending_pair)
                    if INTERLEAVE and idx >= 2:
                        pump_attention(1)
                if idx % 2 == 1:
                    deferred = pair_state.pop('tp', [])
                    pending_pair = {TORDER[idx - 1], t}
            for u in deferred:
                u()
            tiles_done.update(pending_pair)
            # drain the rest of the attention + out-proj stream
            while True:
                if pump_attention(1 << 30) == 0:
                    break
    nc.compile()
    return nc


_NC_CACHE = None


def _get_nc():
    global _NC_CACHE
    if _NC_CACHE is None:
        _NC_CACHE = build()
    return _NC_CACHE


def prep_inputs(x, w_qkv, w_out):
    """Host-side sharding/layout prep. Returns list of per-core input maps."""
    wnp = ml_dtypes.bfloat16
    x2d = np.asarray(x, dtype=np.float32).reshape(L, D)
    w_qkv = np.asarray(w_qkv, dtype=np.float32)
    w_out = np.asarray(w_out, dtype=np.float32)

    # xt[t, c, p, l] = x2d[t*128 + l, c*128 + p]
    # [t, p(hid), c, l] so each per-tile DMA is one linear stream
    xt = np.ascontiguousarray(
        x2d.reshape(NT, LT, NHC, HC).transpose(0, 3, 2, 1)).astype(wnp)

    # rope coefficient tables
    inv_freq = 1.0 / (ROPE_BASE ** (np.arange(0, HD, 2, dtype=np.float64) / HD))
    pos = np.arange(L, dtype=np.float64)[:, None]
    ang = pos * inv_freq[None, :]                       # [L, 64]
    cos, sin = np.cos(ang), np.sin(ang)
    w1 = np.zeros((L, HD), dtype=np.float32)
    w2 = np.zeros((L, HD), dtype=np.float32)
    w1[:, 0::2] = -sin
    w1[:, 1::2] = cos
    w2[:, 0::2] = cos
    w2[:, 1::2] = sin
    w1 = np.ascontiguousarray(w1.reshape(NT, LT, HD).transpose(1, 0, 2))
    w2 = np.ascontiguousarray(w2.reshape(NT, LT, HD).transpose(1, 0, 2))

    # causal mask tile for diagonal blocks
    i = np.arange(128)[:, None]
    j = np.arange(128)[None, :]
    mask4 = np.where(i <= j, 0.0, NEG).astype(np.float32)  # [128, 128]
    ident = np.eye(128, dtype=np.float32).astype(wnp)

    in_maps = []
    for c in range(N_CORES):
        h0 = HPC * c
        rows = []
        for part in range(3):  # q, k, v
            for hh in range(HPC):
                base = part * D + (h0 + hh) * HD
                rows.append(w_qkv[base:base + HD])
        w_c = np.concatenate(rows, axis=0)              # [768, D]
        wt = np.ascontiguousarray(w_c.T).astype(wnp)    # [D, 768]
        wo = np.ascontiguousarray(
            w_out[:, h0 * HD:(h0 + HPC) * HD].T.reshape(HPC, HD, D)
            .transpose(1, 0, 2)).astype(wnp)
        in_maps.append({
            "xt": xt, "wt": wt, "wo": wo, "w1": w1, "w2": w2,
            "mask4": mask4, "ident": ident,
        })
    return in_maps


def kernel(x, w_qkv, w_out, mask, _trace=False):
    """Full MHA forward. Returns [1, L, D] float32."""
    nc = _get_nc()
    in_maps = prep_inputs(x, w_qkv, w_out)
    res = run_bass_kernel_spmd(nc, in_maps, core_ids=list(range(N_CORES)),
                               trace=_trace)
    acc = np.zeros((L, D), dtype=np.float32)
    for r in res.results:
        acc += np.asarray(r["out"], dtype=np.float32)
    out = acc.reshape(1, L, D)
    if _trace:
        return out, res
    return out


# revision 52
# speedup vs baseline: 1.0295x; 1.0295x over previous
"""Trainium2 Bass kernel for 16-head causal MHA (RMSNorm+RoPE on q,k).

Tensor-parallel over heads: 8 cores x 2 heads each. Each core computes
qkv projection for its heads, norm+rope, causal attention, and a partial
out-projection; the host sums the 8 partial outputs.

v4 notes (on top of the v2 exp-stationary PV design):
- Scores are computed transposed [k, q]; exp tiles serve as the
  STATIONARY matmul operand for PV with a ones-column appended to V, so
  the PV output lands as [q, v|den]: softmax denominator is column 128
  and the division is a per-partition scalar multiply.
- Single fused pipeline: attention group g (q rows [512g, 512g+512)) is
  emitted interleaved with phase-1 tiles >= 4g+4, so PE never drains at
  the phase boundary and the exp/ACT load spreads over the whole span.
- ACT runs ONLY Exp + table-free copies (Square/Sqrt thrash the exp
  activation table at ~1.3us per reload). The RMSNorm 1/sqrt(m) runs on
  DVE as a 2-step Newton iteration seeded with the Taylor line
  1.5 - m/2 (m = mean-square concentrates near 1), batched per 2 tiles.
- The k-side norm scale commutes through RoPE and folds into the exp's
  per-partition scale AP, so only q is scaled explicitly.
- PSUM budget (8 banks): p_qk 2 + p_v 1 + {scores, p_y, transposes}
  shared 3-buf tag 3 + p_o 2.
"""
import os
import ml_dtypes
import numpy as np

import concourse.bacc as bacc
import concourse.mybir as mybir
import concourse.tile as tile
from concourse.ap import AP
from concourse.bass_utils import run_bass_kernel_spmd


def _bcast_mid(ap2d, n):
    """[128, X] -> [128, n, X] with step-0 middle dim."""
    return AP(tensor=ap2d.tensor, offset=ap2d.offset,
              ap=[list(ap2d.ap[0]), [0, n], list(ap2d.ap[1])])

F32 = mybir.dt.float32
BF16 = mybir.dt.bfloat16
WDT = BF16
AF = mybir.ActivationFunctionType
ALU = mybir.AluOpType
AX = mybir.AxisListType

N_CORES = 8
L = 2048
D = 2048
HD = 128
N_HEAD = 16
HPC = N_HEAD // N_CORES  # heads per core = 2
LT = 128                 # L-tile rows
NT = L // LT             # 16 L-tiles
HC = 128                 # hid chunk
NHC = D // HC            # 16 hid chunks
QT = 512                 # q-tile width in attention
NQT = L // QT            # 4
VW = 130                 # v row stride (128 dims + ones col + pad)
EPS = 1e-5
ROPE_BASE = 10000.0
SCALE = 1.0 / float(np.sqrt(HD))
NEG = -1.0e9
MSCALE = 2.75            # rsqrt range normalizer: m*MSCALE lands in [0.5, 2]
INTERLEAVE = os.environ.get("MHA_INTERLEAVE", "1") == "1"
NEWTON = os.environ.get("MHA_NEWTON", "1") == "1"
TORDER = list(range(NT))
# attention q-groups (first q-tile, n q-tiles): g0-g2 full 512-wide; the
# tail group is split in two 256-wide halves so the first half unblocks
# at tile pair (12,13) and the serial tail is only q-tiles 14-15.
GROUPS = [(0, 4), (4, 4), (8, 4), (12, 2), (14, 2)]


def build():
    nc = bacc.Bacc("TRN2", target_bir_lowering=False, debug=False,
                   enable_asserts=False, num_devices=N_CORES)

    # Per-core external inputs (host-prepped layouts; see prep_inputs()).
    xt = nc.dram_tensor("xt", [NT, HC, NHC, LT], WDT, kind="ExternalInput")
    wt = nc.dram_tensor("wt", [D, 6 * HD], WDT, kind="ExternalInput")
    wo = nc.dram_tensor("wo", [HD, HPC, D], WDT, kind="ExternalInput")
    w1 = nc.dram_tensor("w1", [LT, NT, HD], F32, kind="ExternalInput")
    w2 = nc.dram_tensor("w2", [LT, NT, HD], F32, kind="ExternalInput")
    mask4 = nc.dram_tensor("mask4", [128, 128], F32, kind="ExternalInput")
    ident_in = nc.dram_tensor("ident", [128, 128], WDT, kind="ExternalInput")

    # partial outputs travel bf16: host-side sum of 8 bf16 partials adds
    # ~2.3e-3 rel err (tolerance 2e-2) and halves evac + output DMA cost
    out = nc.dram_tensor("out", [L, D], WDT, kind="ExternalOutput")

    with tile.TileContext(nc) as tc:
        with (
            tc.tile_pool(name="const", bufs=1) as constp,
            tc.tile_pool(name="wpool", bufs=1) as wpool,
            tc.tile_pool(name="persist", bufs=1) as persist,
            tc.tile_pool(name="xin", bufs=7) as xin,
            tc.tile_pool(name="qkv", bufs=3) as qkvp,
            tc.tile_pool(name="attn", bufs=4) as attnp,
            tc.tile_pool(name="res", bufs=4) as resp,
            tc.tile_pool(name="ps_qkv", bufs=1, space="PSUM") as ps_pv,
            tc.tile_pool(name="ps_s", bufs=3, space="PSUM") as ps_sc,
            tc.tile_pool(name="ps_o", bufs=1, space="PSUM") as ps_po,
        ):
            # ---- startup DMA order: first matmul needs x tile 0 + w
            # chunk 0 only; issue those first, stream the rest behind.
            w_sb = wpool.tile([128, NHC, 6 * HD], WDT)
            x_first = xin.tile([128, NHC, LT], WDT, tag="x", name="x_tile")
            nc.sync.dma_start(out=x_first, in_=xt[0, :, :, :])
            for c in range(4):
                eng = nc.scalar if c % 2 == 0 else nc.sync
                eng.dma_start(out=w_sb[:, c, :],
                              in_=wt[c * 128:(c + 1) * 128, :])
            w1_sb = constp.tile([128, NT, HD], F32)
            w2_sb = constp.tile([128, NT, HD], F32)
            nc.scalar.dma_start(out=w1_sb[:, 0:2, :], in_=w1[:, 0:2, :])
            nc.sync.dma_start(out=w2_sb[:, 0:2, :], in_=w2[:, 0:2, :])
            x_1 = xin.tile([128, NHC, LT], WDT, tag="x", name="x_tile")
            nc.sync.dma_start(out=x_1, in_=xt[1, :, :, :])
            x_2 = xin.tile([128, NHC, LT], WDT, tag="x", name="x_tile")
            nc.scalar.dma_start(out=x_2, in_=xt[2, :, :, :])
            for c in range(4, NHC):
                eng = nc.scalar if c % 2 == 0 else nc.sync
                eng.dma_start(out=w_sb[:, c, :],
                              in_=wt[c * 128:(c + 1) * 128, :])
            nc.scalar.dma_start(out=w1_sb[:, 2:6, :], in_=w1[:, 2:6, :])
            nc.sync.dma_start(out=w2_sb[:, 2:6, :], in_=w2[:, 2:6, :])
            mask_sb = constp.tile([128, 128], F32)
            nc.gpsimd.dma_start(out=mask_sb, in_=mask4[:, :])
            ident = constp.tile([128, 128], WDT)
            nc.gpsimd.dma_start(out=ident, in_=ident_in[:, :])
            wo_sb = wpool.tile([128, HPC, D], WDT)
            eps_sb = constp.tile([128, 1], F32)
            nc.vector.memset(eps_sb, EPS)

            # persistent activations
            # v_sb: [kpos-part, t, head, 130]; col 128 is the ones column
            # feeding the softmax denominator, col 129 is alignment pad.
            v_sb = persist.tile([128, NT, HPC, VW], WDT)
            nc.vector.memset(v_sb[:, :, :, 128:VW], 1.0)
            qT = persist.tile([128, HPC, L], WDT)               # [d, h, L]
            kT = persist.tile([128, HPC, L], WDT)
            # per-kpos exp scale: SCALE / sqrt(mean k^2 + eps), per head
            sk_sb = persist.tile([128, NT, HPC], F32)



            xs = {0: x_first, 1: x_1, 2: x_2}

            def load_x(t):
                x_tl = xin.tile([128, NHC, LT], WDT, tag="x", name="x_tile")
                nc.gpsimd.dma_start(out=x_tl, in_=xt[t, :, :, :])
                xs[t] = x_tl

            for i in range(3, 6):
                load_x(TORDER[i])

            # ---------------- phase 1 unit generator ------------------
            pair_state = {}

            def phase1_units(idx, t):
                x_tile = xs.pop(t)
                p_qk = ps_pv.tile([128, 4 * HD], F32, tag="pqk", bufs=2,
                                  name="p_qk")
                p_v = ps_pv.tile([128, HPC * HD], F32, tag="pv", bufs=1,
                                 name="p_v")

                def chunk(c):
                    nc.tensor.matmul(p_qk, x_tile[:, c, :],
                                     w_sb[:, c, 0:4 * HD],
                                     start=(c == 0), stop=(c == NHC - 1))
                    nc.tensor.matmul(p_v, x_tile[:, c, :],
                                     w_sb[:, c, 4 * HD:6 * HD],
                                     start=(c == 0), stop=(c == NHC - 1))
                for c in range(NHC):
                    yield lambda c=c: chunk(c)

                def post_a():
                    if idx == 1:
                        nc.scalar.dma_start(out=w1_sb[:, 6:NT, :],
                                            in_=w1[:, 6:NT, :])
                        nc.sync.dma_start(out=w2_sb[:, 6:NT, :],
                                          in_=w2[:, 6:NT, :])
                    if idx == 4:
                        nc.scalar.dma_start(out=wo_sb, in_=wo[:, :, :])
                    if idx + 6 < NT:
                        load_x(TORDER[idx + 6])
                    # v out of psum (ACT copy: table-free)
                    nc.scalar.copy(
                        v_sb[:, t, :, 0:HD],
                        p_v.rearrange("p (h d) -> p h d", h=HPC))
                    # rope RAW q,k straight out of PSUM (norm commutes
                    # through rope); fp32 in, bf16 out, de-interleaved
                    roped = qkvp.tile([128, 4 * HD], WDT, tag="roped",
                                      bufs=4, name="roped")
                    roped4 = roped.rearrange("p (g h x) -> p g h x", g=4, h=2)
                    for half, wtab in ((0, w1_sb), (1, w2_sb)):
                        z = qkvp.tile([128, 4 * HD], F32, tag="z", name="z")
                        nc.vector.tensor_mul(
                            z.rearrange("p (g d) -> p g d", g=4),
                            p_qk.rearrange("p (g d) -> p g d", g=4),
                            _bcast_mid(wtab[:, t, :], 4))
                        with nc.allow_low_precision("2-elem rope pairs"):
                            nc.vector.reduce_sum(
                                roped4[:, :, half, :],
                                z.rearrange("p (g x two) -> p g x two",
                                            g=4, two=2),
                                axis=AX.X)
                    if NEWTON:
                        # ssum[g] = MSCALE*mean_d roped[g,d]^2 (rope
                        # preserves row norms); batched per tile pair
                        if t % 2 == 0:
                            ss = qkvp.tile([128, 8], F32, tag="ssum",
                                           name="ssum")
                            pair_state['ssum'] = ss
                        ss = pair_state['ssum']
                        off = (t % 2) * 4
                        sq = qkvp.tile([128, 4 * HD], WDT, tag="sq",
                                       name="sq")
                        nc.vector.tensor_mul(sq, roped, roped)
                        nc.vector.reduce_sum(
                            ss[:, off:off + 4],
                            sq.rearrange("p (g d) -> p g d", g=4), axis=AX.X)
                        nc.vector.tensor_scalar_mul(
                            ss[:, off:off + 4], ss[:, off:off + 4],
                            MSCALE / HD)
                    else:
                        # baseline norm chain: ACT square + DVE reduce +
                        # ACT sqrt(+eps) + DVE recip, then scale+transpose
                        sq = qkvp.tile([128, 4 * HD], F32, tag="sq",
                                       name="sq")
                        nc.scalar.activation(sq, roped, AF.Square)
                        ssb = qkvp.tile([128, 4], F32, tag="ssb", name="ssb")
                        nc.vector.reduce_sum(
                            ssb, sq.rearrange("p (g d) -> p g d", g=4),
                            axis=AX.X)
                        nc.scalar.activation(ssb, ssb, AF.Sqrt,
                                             scale=1.0 / HD, bias=eps_sb)
                        s_val = qkvp.tile([128, 4], F32, tag="sval",
                                          name="s_val")
                        nc.vector.reciprocal(s_val, ssb)
                        finish_tile(t, roped, s_val, 0)
                    pair_state[idx % 2] = roped
                yield post_a

                def finish_tile(tt, roped, y, soff):
                    # scale q,k segs by 1/sqrt(m) in place (DVE); the PE
                    # transposes are deferred into the next tile's stream
                    # so they don't head-of-line block behind this chain
                    roped2 = roped.rearrange("p (g d) -> p g d", g=4)
                    nc.vector.tensor_mul(
                        roped2, roped2,
                        y[:, soff:soff + 4].to_broadcast([128, 4, HD]))

                    def tp_unit(seg):
                        tgt = qT if seg < 2 else kT
                        h = seg % 2
                        p_tr = ps_sc.tile([128, 128], WDT, tag="sc",
                                          name="p_tr")
                        nc.tensor.transpose(
                            p_tr, roped[:, seg * HD:(seg + 1) * HD], ident)
                        nc.scalar.copy(tgt[:, h, tt * LT:(tt + 1) * LT],
                                       p_tr)
                    pair_state.setdefault('tp', []).extend(
                        (lambda seg=seg: tp_unit(seg)) for seg in range(4))

                if NEWTON and t % 2 == 1:
                    def newton():
                        # DVE rsqrt for the pair: ss holds m' = MSCALE*m
                        # (m' in ~[0.5, 2.0]); seed y0 = (1 + 1/m')/2
                        # (<=10% err there), 2 Newton steps -> <=4e-4;
                        # the sqrt(MSCALE) unscale folds into the last mul.
                        # Single-scalar DVE ops only (2-imm tensor_scalar /
                        # scalar_tensor_tensor mis-execute on HW).
                        ss = pair_state['ssum']
                        y = qkvp.tile([128, 8], F32, tag="rs_y", name="y_rs")
                        r = qkvp.tile([128, 8], F32, tag="rs_r", name="r_rs")
                        nc.vector.reciprocal(r, ss)
                        nc.vector.tensor_scalar_add(r, r, 1.0)
                        nc.vector.tensor_scalar_mul(y, r, 0.5)
                        for it in range(2):
                            u = qkvp.tile([128, 8], F32, tag="rs_u",
                                          name="u_rs")
                            nc.vector.tensor_mul(u, y, y)
                            nc.vector.tensor_mul(u, u, ss)
                            nc.vector.tensor_scalar_mul(u, u, -0.5)
                            nc.vector.tensor_scalar_add(u, u, 1.5)
                            nc.vector.tensor_mul(y, y, u)
                        nc.vector.tensor_scalar_mul(y, y, float(MSCALE ** 0.5))
                        return y
                    holder = {}

                    def post_b0():
                        y = holder.setdefault('y', newton())
                        finish_tile(t - 1, pair_state[0], y, 0)

                    def post_b1():
                        y = holder['y']
                        finish_tile(t, pair_state[1], y, 4)
                    yield post_b0
                    yield post_b1

            # --------------- attention unit generator -----------------
            # p_o packs the group's q-chunks of [v(128)|den(1)] into PSUM.
            # The first chunk of each bank carries start=True (clears the
            # bank's has_written bits); later chunks in the same bank
            # overwrite-on-clear-bit at kc==0.
            PO_OFF = (0, 130, 260, 512)
            PO_START = (0, 3)  # qc's whose kc==0 matmul is the bank-first

            def head_units(t0, ntq, h, oT_tiles):
                """Yields (need_tiles, unit) for q rows [128*t0, ...)."""
                W = ntq * 128
                nkc = t0 + ntq
                qtiles = frozenset(range(t0, t0 + ntq))
                p_o = ps_po.tile([128, 1024], F32, tag="po", name="p_o")
                exps = {}

                def score(kc):
                    diag = kc >= t0
                    q0 = (kc - t0) * 128 if diag else 0
                    p_s = ps_sc.tile([128, QT], F32, tag="sc", name="p_s")
                    nc.tensor.matmul(
                        p_s[:, q0:W], kT[:, h, kc * 128:(kc + 1) * 128],
                        qT[:, h, t0 * 128 + q0:t0 * 128 + W],
                        start=True, stop=True)
                    if diag:
                        nc.vector.tensor_add(
                            p_s[:, q0:q0 + 128], p_s[:, q0:q0 + 128], mask_sb)
                    expT = attnp.tile([128, QT], WDT, tag="expT", bufs=6,
                                      name="expT")
                    nc.scalar.activation(expT[:, q0:W], p_s[:, q0:W],
                                         AF.Exp, scale=SCALE)
                    exps[kc] = expT

                def pv(kc):
                    expT = exps.pop(kc)
                    for qc in range(max(0, kc - t0), ntq):
                        off = PO_OFF[qc]
                        nc.tensor.matmul(
                            p_o[:, off:off + 129],
                            expT[:, qc * 128:(qc + 1) * 128],
                            v_sb[:, kc, h, 0:129],
                            start=(kc == 0 and qc in PO_START),
                            stop=(kc == t0 + qc),
                            skip_group_check=True)

                yield qtiles | {0}, lambda: score(0)
                for kc in range(1, nkc):
                    yield (qtiles | {kc},
                           lambda kc=kc: (score(kc), pv(kc - 1)))
                yield qtiles, lambda: pv(nkc - 1)

                # o_sb[q, vd] = p_o[q, 0:128] / den (den = col 128), then
                # transpose back to [vd, q] on PE for the out-projection
                def division(qc):
                    off = PO_OFF[qc]
                    inv = attnp.tile([128, 1], F32, tag="inv", bufs=4,
                                     name="inv")
                    nc.vector.reciprocal(inv, p_o[:, off + 128:off + 129])
                    o_sb = attnp.tile([128, HD], WDT, tag="osb", bufs=8,
                                      name="o_sb")
                    nc.vector.tensor_scalar_mul(o_sb, p_o[:, off:off + HD],
                                                inv)
                    p_tr = ps_sc.tile([128, 128], WDT, tag="sc", name="p_tr2")
                    nc.tensor.transpose(p_tr, o_sb, ident)
                    oT = resp.tile([128, 128], WDT, tag="oT", bufs=40,
                                   name="oT")
                    nc.scalar.copy(oT, p_tr)
                    oT_tiles[(h, qc)] = oT
                for qc in range(ntq):
                    yield qtiles, lambda qc=qc: division(qc)

            def outproj_units(t0, ntq, oT_tiles):
                for tt in range(ntq):
                    t = t0 + tt
                    for ec in range(4):
                        def u(t=t, tt=tt, ec=ec):
                            p_y = ps_sc.tile([128, QT], F32, tag="sc",
                                             name="p_y")
                            nc.tensor.matmul(
                                p_y, oT_tiles[(0, tt)],
                                wo_sb[:, 0, ec * 512:(ec + 1) * 512],
                                start=True, stop=False)
                            nc.tensor.matmul(
                                p_y, oT_tiles[(1, tt)],
                                wo_sb[:, 1, ec * 512:(ec + 1) * 512],
                                start=False, stop=True)
                            y = resp.tile([128, QT], WDT, tag="y", bufs=4,
                                          name="y")
                            if ec % 2 == 0:
                                nc.scalar.copy(y, p_y)
                            else:
                                nc.vector.tensor_copy(y, p_y)
                            eng = nc.gpsimd if ec % 2 == 0 else nc.sync
                            eng.dma_start(
                                out=out[t * LT:(t + 1) * LT,
                                        ec * 512:(ec + 1) * 512],
                                in_=y)
                        yield u

            def attention_units():
                """Yields (need_tiles, unit). Out-proj units of early
                groups are DEFERRED and woven into the later (exp-bound,
                ACT-limited) groups so PE stays dense there."""
                backlog = []
                for gi, (t0, ntq) in enumerate(GROUPS):
                    oT_tiles = {}
                    last = gi == len(GROUPS) - 1
                    # for the last group, weave its own out-proj directly
                    # behind each h1 division so the tail doesn't serialize
                    op3 = (list(outproj_units(t0, ntq, oT_tiles))
                           if last else None)
                    for h in range(HPC):
                        hu = list(head_units(t0, ntq, h, oT_tiles))
                        nd = len(hu) - ntq  # index of first division unit
                        for i, (need, u) in enumerate(hu):
                            yield need, u
                            if last and h == 1 and i >= nd:
                                for uu in op3[(i - nd) * 4:(i - nd + 1) * 4]:
                                    yield frozenset(), uu
                            elif gi >= 2 and backlog:
                                yield frozenset(), backlog.pop(0)
                    if not last:
                        backlog.extend(outproj_units(t0, ntq, oT_tiles))
                for n in backlog:
                    yield frozenset(), n

            # ---------------- fused emission schedule -----------------
            attn = attention_units()
            pending = None  # (need, unit) fetched but not yet eligible
            tiles_done = set()

            def pump_attention(budget):
                """Emit up to `budget` eligible attention units."""
                nonlocal pending
                emitted = 0
                while emitted < budget:
                    if pending is None:
                        pending = next(attn, None)
                        if pending is None:
                            return emitted
                    need, u = pending
                    if not need <= tiles_done:
                        return emitted  # inputs not emitted yet
                    pending = None
                    u()
                    emitted += 1
                return emitted

            deferred = []     # previous pair's transpose units
            pending_pair = set()
            for idx, t in enumerate(TORDER):
                units = list(phase1_units(idx, t))
                for u in units:
                    u()
                    if deferred:
                        deferred.pop(0)()
                        if not deferred:
                            # pair fully emitted (incl. qT/kT transposes)
                            tiles_done.update(pending_pair)
                    if INTERLEAVE and idx >= 2:
                        pump_attention(1)
                if idx % 2 == 1:
                    deferred = pair_state.pop('tp', [])
                    pending_pair = {TORDER[idx - 1], t}
            for u in deferred:
                u()
            tiles_done.update(pending_pair)
            # drain the rest of the attention + out-proj stream
            while True:
                if pump_attention(1 << 30) == 0:
                    break
    nc.compile()
    return nc


_NC_CACHE = None


def _get_nc():
    global _NC_CACHE
    if _NC_CACHE is None:
        _NC_CACHE = build()
    return _NC_CACHE


def prep_inputs(x, w_qkv, w_out):
    """Host-side sharding/layout prep. Returns list of per-core input maps."""
    wnp = ml_dtypes.bfloat16
    x2d = np.asarray(x, dtype=np.float32).reshape(L, D)
    w_qkv = np.asarray(w_qkv, dtype=np.float32)
    w_out = np.asarray(w_out, dtype=np.float32)

    # xt[t, c, p, l] = x2d[t*128 + l, c*128 + p]
    # [t, p(hid), c, l] so each per-tile DMA is one linear stream
    xt = np.ascontiguousarray(
        x2d.reshape(NT, LT, NHC, HC).transpose(0, 3, 2, 1)).astype(wnp)

    # rope coefficient tables
    inv_freq = 1.0 / (ROPE_BASE ** (np.arange(0, HD, 2, dtype=np.float64) / HD))
    pos = np.arange(L, dtype=np.float64)[:, None]
    ang = pos * inv_freq[None, :]                       # [L, 64]
    cos, sin = np.cos(ang), np.sin(ang)
    w1 = np.zeros((L, HD), dtype=np.float32)
    w2 = np.zeros((L, HD), dtype=np.float32)
    w1[:, 0::2] = -sin
    w1[:, 1::2] = cos
    w2[:, 0::2] = cos
    w2[:, 1::2] = sin
    w1 = np.ascontiguousarray(w1.reshape(NT, LT, HD).transpose(1, 0, 2))
    w2 = np.ascontiguousarray(w2.reshape(NT, LT, HD).transpose(1, 0, 2))

    # causal mask tile for diagonal blocks
    i = np.arange(128)[:, None]
    j = np.arange(128)[None, :]
    mask4 = np.where(i <= j, 0.0, NEG).astype(np.float32)  # [128, 128]
    ident = np.eye(128, dtype=np.float32).astype(wnp)

    in_maps = []
    for c in range(N_CORES):
        h0 = HPC * c
        rows = []
        for part in range(3):  # q, k, v
            for hh in range(HPC):
                base = part * D + (h0 + hh) * HD
                rows.append(w_qkv[base:base + HD])
        w_c = np.concatenate(rows, axis=0)              # [768, D]
        wt = np.ascontiguousarray(w_c.T).astype(wnp)    # [D, 768]
        wo = np.ascontiguousarray(
            w_out[:, h0 * HD:(h0 + HPC) * HD].T.reshape(HPC, HD, D)
            .transpose(1, 0, 2)).astype(wnp)
        in_maps.append({
            "xt": xt, "wt": wt, "wo": wo, "w1": w1, "w2": w2,
            "mask4": mask4, "ident": ident,
        })
    return in_maps


def kernel(x, w_qkv, w_out, mask, _trace=False):
    """Full MHA forward. Returns [1, L, D] float32."""
    nc = _get_nc()
    in_maps = prep_inputs(x, w_qkv, w_out)
    res = run_bass_kernel_spmd(nc, in_maps, core_ids=list(range(N_CORES)),
                               trace=_trace)
    acc = np.zeros((L, D), dtype=np.float32)
    for r in res.results:
        acc += np.asarray(r["out"], dtype=np.float32)
    out = acc.reshape(1, L, D)
    if _trace:
        return out, res
    return out
